# revision 22
# baseline (speedup 1.0000x reference)
"""BalancedWeightClusterLoss on 8 Trainium2 NeuronCores (Bass/Tile).

Reference computation (per channel c of weight [C, K], scale [C]):
    mean, std(ddof=1) over K
    lower = mean - 2*std ; step = 4*std/15
    idx = clip((w - lower)/step, 0, 14) -> int (trunc == floor here)
    target = scale * (idx - 7)
    loss = sum |w - target|

Kernel derivation (per channel; r = 1/step, nb1 = mean*r - 7):
    idx = floor((w-lower)*r) = round(w*r - nb1)      (round(x-.5)==floor(x))
    jc7 = clip(round(z), 0, 14) - 7,  z = w*r - nb1
    loss = sum |w - s*jc7|

Engine split (both ~26us/block, pipelined across 4 row-blocks):
    ACT:  Copy(w_f32 -> w_f16) + riding accumulator => Sum(w)
          Square(w_f32, cols [0:SQA]) + accumulator => most of Sum(w^2)
          Sqrt for step (tiny)
    DVE:  SQSUM custom op on cols [SQA:K] of w_f16  => rest of Sum(w^2)
          z = w_f16*r - nb1     (tensor_scalar dual, f16, 4x mode)
          VQ_LOSS_ANT custom op (registered at import):
             j = min(max(z + 2^23, 2^23) - (2^23+7), 7)  [f32 internal round]
             out = |w - s*j|, accum_out = per-channel loss sum
          (1 elem/cycle; the loss reduction rides the same pass for free)
    All big reductions ride engine accumulators; no separate reduce passes.
    Square-pass scratch and custom-op garbage both land in the z tile.
    Emission is software-pipelined (phase1 of block b+1 before phase2 of
    block b) and block 0 is finely chunked so its stats trail the DMA.

Sharding: channels 4096 -> 512 per core (8 cores) x 4 row-blocks of 128
partitions. w is read from HBM exactly once (memory roofline ~94us/core).
Host sums the 8 x [128, 4] partial losses in float64.
"""
import numpy as np

import concourse.bacc as bacc
import concourse.tile as tile
from concourse import mybir
from concourse.bass_utils import run_bass_kernel_spmd

f32 = mybir.dt.float32
f16 = mybir.dt.float16
Alu = mybir.AluOpType
Act = mybir.ActivationFunctionType

# problem shape (hardcoded per contest contract)
CFULL, K = 4096, 16384
NCORES = 8
CSH = CFULL // NCORES          # 512 channels per core
P = 128                        # SBUF partitions
NBLK = CSH // P                # 4 row-blocks per core
CH = 4096                      # phase-1 chunk (f32 DMA + ACT passes)
NCH = K // CH                  # 4
SQA = 13312                    # Sum(w^2): ACT takes [0:SQA], DVE [SQA:K]

RND = float(2 ** 23)           # f32 round-to-int bias
RND7 = float(2 ** 23 + 7)
INV_K = 1.0 / K
# step^2 = K2 * var_biased ; var_b = E[w^2] - mean^2
K2 = (4.0 / 15.0) ** 2 * (K / (K - 1.0))

_PROGRAM = None


def _vq_ref(in0, in1, c0, c1, c2):
    """numpy reference for VQ_LOSS_ANT (CoreSim executes this)."""
    z32 = np.asarray(in0, np.float32)
    v = (z32 + np.float32(c0)).astype(np.float32)
    v2 = np.maximum(v, np.float32(c0))
    j0 = (v2 - np.float32(c2)).astype(np.float32)
    j = np.minimum(j0, np.float32(c2 - c0))
    t = (j * np.asarray(c1, np.float32)).astype(np.float32)
    ae = np.abs(np.asarray(in1, np.float32) - t)
    return ae, ae.sum(axis=1, keepdims=True)


def _sq_ref(in0, in1, c0, c1, c2):
    """numpy reference for SQSUM_ANT."""
    x = np.asarray(in0, np.float32)
    sq = x * x
    return sq, sq.sum(axis=1, keepdims=True)


def _register_ops():
    """Register the custom DVE ops in concourse's table (runtime append;
    the uop programs are compiled into the per-NEFF DVE table)."""
    import concourse.dve_ops as D
    from concourse.dve_spec import (
        Spec, Src0, Src1, C0, C1, C2, maxx, minn, sq, Bin, AluOp, lower,
        _has_src1,
    )
    from concourse.dve_uop import DveOpSpec

    def reg(name, spec):
        if name in D._SUB_OPCODE_FOR_NAME:
            for op in D.OPS:
                if op.name == name:
                    return op
        row = D._CUSTOM_DVE_ROW_BASE + len(D.OPS)
        assert row < 0x20, "custom DVE row overflow"
        shas = {}
        for ver in ("v3", "v4"):
            s = DveOpSpec(name=name, opcode=row,
                          uops=lower(spec, ver=ver), rd1_en=_has_src1(spec))
            shas[ver] = s.sha(ver)
        op = D.DveOp(name, spec, subdim=False, uops_sha=shas)
        D.OPS.append(op)
        D._SUB_OPCODE_FOR_NAME[name] = row
        D.CUSTOM_DVE_SPECS[name] = spec
        return op

    v = Src0 + C0              # 2^23 + round(z)   (f32 internal)
    v2 = maxx(v, C0)           # clip low: round(z) >= 0
    j0 = v2 - C2               # max(round(z),0) - 7
    j = minn(j0, C2 - C0)      # min(..., 7)  (C2-C0 = 7, auto-hoisted)
    t = j * C1                 # s * jc7
    ae = Bin(AluOp.ABSOLUTE_DIFF, Src1, t)   # |w - s*jc7|
    vq = reg("VQ_LOSS_ANT",
             Spec(body=ae, accum=AluOp.ADD, reference=_vq_ref))
    sqs = reg("SQSUM_ANT",
              Spec(body=sq(Src0), accum=AluOp.ADD, reference=_sq_ref))
    return vq, sqs


def _build():
    vq, sqs = _register_ops()
    nc = bacc.Bacc("TRN2", target_bir_lowering=False, debug=False,
                   num_devices=NCORES)
    w_ext = nc.dram_tensor("w", [CSH, K], f32, kind="ExternalInput")
    s_ext = nc.dram_tensor("s", [CSH, 1], f32, kind="ExternalInput")
    out_ext = nc.dram_tensor("out", [P, NBLK], f32, kind="ExternalOutput")

    with tile.TileContext(nc) as tc:
        with (
            tc.tile_pool(name="w32p", bufs=2) as w32p,
            tc.tile_pool(name="w16p", bufs=2) as w16p,
            tc.tile_pool(name="zp", bufs=2) as zp,
            tc.tile_pool(name="minis", bufs=2) as minis,
            tc.tile_pool(name="outp", bufs=1) as outp,
        ):
            out_sb = outp.tile([P, NBLK], f32)
            seven = outp.tile([P, 1], f32)
            nc.vector.memset(seven[:], 7.0)

            state = {}

            def phase1(b):
                """DMA + conversion + stats passes + per-channel params."""
                rows = slice(b * P, (b + 1) * P)
                sblk = minis.tile([P, 1], f32, tag="sblk")
                nc.sync.dma_start(sblk[:], s_ext[rows, :])

                w16a = w16p.tile([P, K // 2], f16, tag="w16a")
                w16b = w16p.tile([P, K // 2], f16, tag="w16b")
                wh = [w16a, w16b]
                z = zp.tile([P, K // 2], f16, tag="z")
                z2 = zp.tile([P, K // 2], f16, tag="z2")
                zh = [z, z2]
                # accumulator slots: su in st[0:6], sq in st[6:11], sq-DVE
                # remainder in st[11]
                st = minis.tile([P, 12], f32, tag="st")
                si = 0
                qi = 0
                # w32 tiles span 8192 cols, each filled by 4096-col DMAs
                # (fine-grained DMA rotation, coarse ACT instructions).
                for h in range(2):
                    base = h * (K // 2)
                    w32 = w32p.tile([P, K // 2], f32, tag="w32")
                    if b == 0:
                        # fast pipeline start: chunked so stats trail the
                        # DMA as closely as possible
                        cuts = [0, 2048, 4096, 8192] if h == 0 else \
                               [0, 4096, 8192]
                    else:
                        cuts = [0, CH, 2 * CH]
                    for lo, hi2 in zip(cuts[:-1], cuts[1:]):
                        nc.sync.dma_start(w32[:, lo:hi2],
                                          w_ext[rows, base + lo:base + hi2])
                    # conversion pass carries Sum(w)
                    if b == 0:
                        for lo, hi2 in zip(cuts[:-1], cuts[1:]):
                            nc.scalar.activation(
                                wh[h][:, lo:hi2], w32[:, lo:hi2], Act.Copy,
                                accum_out=st[:, si:si + 1])
                            si += 1
                    else:
                        nc.scalar.activation(
                            wh[h][:], w32[:], Act.Copy,
                            accum_out=st[:, si:si + 1])
                        si += 1
                    # square pass carries Sum(w^2) for [0:SQA]; scratch
                    # lands in the z half (overwritten by ts-z later)
                    hi = min(SQA, base + K // 2) - base
                    if hi > 0:
                        if b == 0:
                            for lo, hi2 in zip(cuts[:-1], cuts[1:]):
                                if lo >= hi:
                                    break
                                nc.scalar.activation(
                                    zh[h][:, lo:min(hi2, hi)],
                                    w32[:, lo:min(hi2, hi)], Act.Square,
                                    accum_out=st[:, 6 + qi:7 + qi])
                                qi += 1
                        else:
                            nc.scalar.activation(
                                zh[h][:, 0:hi], w32[:, 0:hi], Act.Square,
                                accum_out=st[:, 6 + qi:7 + qi])
                            qi += 1
                # remainder of Sum(w^2) on DVE from w_f16
                nc.vector._custom_dve(sqs,
                                      out=z2[:, SQA - K // 2:K // 2],
                                      in0=wh[1][:, SQA - K // 2:K // 2],
                                      accum_out=st[:, 11:12])

                # per-channel params: r = 1/step, nb1 = mean*r - 7
                SUQ = minis.tile([P, 2], f32, tag="SUQ")
                nc.vector.tensor_reduce(SUQ[:, 0:1], st[:, 0:si],
                                        mybir.AxisListType.X, Alu.add)
                nc.vector.tensor_reduce(SUQ[:, 1:2], st[:, 6:6 + qi],
                                        mybir.AxisListType.X, Alu.add)
                nc.vector.tensor_tensor(SUQ[:, 1:2], SUQ[:, 1:2],
                                        st[:, 11:12], Alu.add)
                me2 = minis.tile([P, 2], f32, tag="me2")
                nc.vector.tensor_scalar(me2[:], SUQ[:], INV_K, None,
                                        Alu.mult)
                mean = me2[:, 0:1]
                E2 = me2[:, 1:2]
                nvar = minis.tile([P, 1], f32, tag="nvar")
                # nvar = mean*mean - E2  (= -var_biased)
                nc.vector.scalar_tensor_tensor(nvar[:], mean, mean,
                                               E2, Alu.mult, Alu.subtract)
                step = minis.tile([P, 1], f32, tag="step")
                # step = sqrt(K2*var_b) = Sqrt(-K2 * nvar)
                nc.scalar.activation(step[:], nvar[:], Act.Sqrt,
                                     bias=0.0, scale=-K2)
                r = minis.tile([P, 1], f32, tag="r")
                nc.vector.reciprocal(r[:], step[:])
                nb1 = minis.tile([P, 1], f32, tag="nb1")
                # nb1 = mean*r - 7
                nc.vector.scalar_tensor_tensor(nb1[:], mean, r[:],
                                               seven[:], Alu.mult,
                                               Alu.subtract)
                state[b] = (sblk, wh, zh, r, nb1)

            def phase2(b):
                """z pass + fused loss; accum rides the custom op."""
                sblk, wh, zh, r, nb1 = state.pop(b)
                am = minis.tile([P, 2], f32, tag="am")
                for h in range(2):
                    nc.vector.tensor_scalar(zh[h][:], wh[h][:], r[:],
                                            nb1[:], Alu.mult, Alu.subtract)
                    nc.vector._custom_dve(vq, out=zh[h][:], in0=zh[h][:],
                                          in1=wh[h][:],
                                          s0=RND, s1=sblk[:], imm2=RND7,
                                          accum_out=am[:, h:h + 1])
                nc.vector.tensor_reduce(out_sb[:, b:b + 1], am[:],
                                        mybir.AxisListType.X, Alu.add)

            # software pipelining: emit phase1(b+1) before phase2(b) so
            # block b+1's stats minis overlap block b's custom ops on DVE
            phase1(0)
            for b in range(1, NBLK):
                phase1(b)
                phase2(b - 1)
            phase2(NBLK - 1)

            nc.sync.dma_start(out_ext[:], out_sb[:])

    nc.compile()
    return nc


def _get_program():
    global _PROGRAM
    if _PROGRAM is None:
        _PROGRAM = _build()
    return _PROGRAM


def kernel(weight, scale):
    w = np.ascontiguousarray(np.asarray(weight, dtype=np.float32))
    s = np.ascontiguousarray(
        np.asarray(scale, dtype=np.float32)).reshape(CFULL, 1)
    assert w.shape == (CFULL, K), w.shape

    nc = _get_program()
    in_maps = [
        {"w": w[i * CSH:(i + 1) * CSH], "s": s[i * CSH:(i + 1) * CSH]}
        for i in range(NCORES)
    ]
    res = run_bass_kernel_spmd(nc, in_maps, list(range(NCORES)))
    total = 0.0
    for i in range(NCORES):
        total += res.results[i]["out"].astype(np.float64).sum()
    return np.float32(total)


# revision 23
# speedup vs baseline: 1.0303x; 1.0303x over previous
"""BalancedWeightClusterLoss on 8 Trainium2 NeuronCores (Bass/Tile).

Reference computation (per channel c of weight [C, K], scale [C]):
    mean, std(ddof=1) over K
    lower = mean - 2*std ; step = 4*std/15
    idx = clip((w - lower)/step, 0, 14) -> int (trunc == floor here)
    target = scale * (idx - 7)
    loss = sum |w - target|

Kernel derivation (per channel; r = 1/step, nb1 = mean*r - 7):
    idx = floor((w-lower)*r) = round(w*r - nb1)      (round(x-.5)==floor(x))
    jc7 = clip(round(z), 0, 14) - 7,  z = w*r - nb1
    loss = sum |w - s*jc7|

Engine split (both ~26us/block, pipelined across 4 row-blocks):
    ACT:  Copy(w_f32 -> w_f16) + riding accumulator => Sum(w)
          Square(w_f32, cols [0:SQA]) + accumulator => most of Sum(w^2)
          Sqrt for step (tiny)
    DVE:  SQSUM custom op on cols [SQA:K] of w_f16  => rest of Sum(w^2)
          z = w_f16*r - nb1     (tensor_scalar dual, f16, 4x mode)
          VQ_LOSS_ANT custom op (registered at import):
             j = min(max(z + 2^23, 2^23) - (2^23+7), 7)  [f32 internal round]
             out = |w - s*j|, accum_out = per-channel loss sum
          (1 elem/cycle; the loss reduction rides the same pass for free)
    All big reductions ride engine accumulators; no separate reduce passes.
    Square-pass scratch and custom-op garbage both land in the z tile.
    Emission is software-pipelined (phase1 of block b+1 before phase2 of
    block b) and block 0 is finely chunked so its stats trail the DMA.

Sharding: channels 4096 -> 512 per core (8 cores) x 4 row-blocks of 128
partitions. w is read from HBM exactly once (memory roofline ~94us/core).
Host sums the 8 x [128, 4] partial losses in float64.
"""
import numpy as np

import concourse.bacc as bacc
import concourse.tile as tile
from concourse import mybir
from concourse.bass_utils import run_bass_kernel_spmd

f32 = mybir.dt.float32
f16 = mybir.dt.float16
Alu = mybir.AluOpType
Act = mybir.ActivationFunctionType

# problem shape (hardcoded per contest contract)
CFULL, K = 4096, 16384
NCORES = 8
CSH = CFULL // NCORES          # 512 channels per core
P = 128                        # SBUF partitions
NBLK = CSH // P                # 4 row-blocks per core
CH = 4096                      # phase-1 chunk (f32 DMA + ACT passes)
NCH = K // CH                  # 4
SQA = 13312                    # Sum(w^2): ACT takes [0:SQA], DVE [SQA:K]

RND = float(2 ** 23)           # f32 round-to-int bias
RND7 = float(2 ** 23 + 7)
INV_K = 1.0 / K
# step^2 = K2 * var_biased ; var_b = E[w^2] - mean^2
K2 = (4.0 / 15.0) ** 2 * (K / (K - 1.0))

_PROGRAM = None


def _vq_ref(in0, in1, c0, c1, c2):
    """numpy reference for VQ_LOSS_ANT (CoreSim executes this)."""
    z32 = np.asarray(in0, np.float32)
    v = (z32 + np.float32(c0)).astype(np.float32)
    v2 = np.maximum(v, np.float32(c0))
    j0 = (v2 - np.float32(c2)).astype(np.float32)
    j = np.minimum(j0, np.float32(c2 - c0))
    t = (j * np.asarray(c1, np.float32)).astype(np.float32)
    ae = np.abs(np.asarray(in1, np.float32) - t)
    return ae, ae.sum(axis=1, keepdims=True)


def _sq_ref(in0, in1, c0, c1, c2):
    """numpy reference for SQSUM_ANT."""
    x = np.asarray(in0, np.float32)
    sq = x * x
    return sq, sq.sum(axis=1, keepdims=True)


def _register_ops():
    """Register the custom DVE ops in concourse's table (runtime append;
    the uop programs are compiled into the per-NEFF DVE table)."""
    import concourse.dve_ops as D
    from concourse.dve_spec import (
        Spec, Src0, Src1, C0, C1, C2, maxx, minn, sq, Bin, AluOp, lower,
        _has_src1,
    )
    from concourse.dve_uop import DveOpSpec

    def reg(name, spec):
        if name in D._SUB_OPCODE_FOR_NAME:
            for op in D.OPS:
                if op.name == name:
                    return op
        row = D._CUSTOM_DVE_ROW_BASE + len(D.OPS)
        assert row < 0x20, "custom DVE row overflow"
        shas = {}
        for ver in ("v3", "v4"):
            s = DveOpSpec(name=name, opcode=row,
                          uops=lower(spec, ver=ver), rd1_en=_has_src1(spec))
            shas[ver] = s.sha(ver)
        op = D.DveOp(name, spec, subdim=False, uops_sha=shas)
        D.OPS.append(op)
        D._SUB_OPCODE_FOR_NAME[name] = row
        D.CUSTOM_DVE_SPECS[name] = spec
        return op

    v = Src0 + C0              # 2^23 + round(z)   (f32 internal)
    v2 = maxx(v, C0)           # clip low: round(z) >= 0
    j0 = v2 - C2               # max(round(z),0) - 7
    j = minn(j0, C2 - C0)      # min(..., 7)  (C2-C0 = 7, auto-hoisted)
    t = j * C1                 # s * jc7
    ae = Bin(AluOp.ABSOLUTE_DIFF, Src1, t)   # |w - s*jc7|
    vq = reg("VQ_LOSS_ANT",
             Spec(body=ae, accum=AluOp.ADD, reference=_vq_ref))
    sqs = reg("SQSUM_ANT",
              Spec(body=sq(Src0), accum=AluOp.ADD, reference=_sq_ref))
    return vq, sqs


def _build():
    vq, sqs = _register_ops()
    nc = bacc.Bacc("TRN2", target_bir_lowering=False, debug=False,
                   num_devices=NCORES)
    w_ext = nc.dram_tensor("w", [CSH, K], f32, kind="ExternalInput")
    s_ext = nc.dram_tensor("s", [CSH, 1], f32, kind="ExternalInput")
    out_ext = nc.dram_tensor("out", [P, NBLK], f32, kind="ExternalOutput")

    with tile.TileContext(nc) as tc:
        with (
            tc.tile_pool(name="w32p", bufs=2) as w32p,
            tc.tile_pool(name="w16p", bufs=2) as w16p,
            tc.tile_pool(name="zp", bufs=2) as zp,
            tc.tile_pool(name="minis", bufs=2) as minis,
            tc.tile_pool(name="outp", bufs=1) as outp,
        ):
            out_sb = outp.tile([P, NBLK], f32)
            seven = outp.tile([P, 1], f32)
            nc.vector.memset(seven[:], 7.0)

            state = {}

            def phase1(b):
                """DMA + conversion + stats passes + per-channel params."""
                rows = slice(b * P, (b + 1) * P)
                sblk = minis.tile([P, 1], f32, tag="sblk")
                nc.sync.dma_start(sblk[:], s_ext[rows, :])

                w16a = w16p.tile([P, K // 2], f16, tag="w16a")
                w16b = w16p.tile([P, K // 2], f16, tag="w16b")
                wh = [w16a, w16b]
                z = zp.tile([P, K // 2], f16, tag="z")
                z2 = zp.tile([P, K // 2], f16, tag="z2")
                zh = [z, z2]
                # accumulator slots: su in st[0:6], sq in st[6:11], sq-DVE
                # remainder in st[11]
                st = minis.tile([P, 12], f32, tag="st")
                si = 0
                qi = 0
                # w32 tiles span 8192 cols, each filled by 4096-col DMAs
                # (fine-grained DMA rotation, coarse ACT instructions).
                for h in range(2):
                    base = h * (K // 2)
                    w32 = w32p.tile([P, K // 2], f32, tag="w32")
                    if b == 0:
                        # fast pipeline start: chunked so stats trail the
                        # DMA as closely as possible
                        cuts = [0, 2048, 4096, 8192] if h == 0 else \
                               [0, 4096, 8192]
                    else:
                        cuts = [0, CH, 2 * CH]
                    for lo, hi2 in zip(cuts[:-1], cuts[1:]):
                        nc.sync.dma_start(w32[:, lo:hi2],
                                          w_ext[rows, base + lo:base + hi2])
                    # conversion pass carries Sum(w)
                    if b == 0:
                        for lo, hi2 in zip(cuts[:-1], cuts[1:]):
                            nc.scalar.activation(
                                wh[h][:, lo:hi2], w32[:, lo:hi2], Act.Copy,
                                accum_out=st[:, si:si + 1])
                            si += 1
                    else:
                        nc.scalar.activation(
                            wh[h][:], w32[:], Act.Copy,
                            accum_out=st[:, si:si + 1])
                        si += 1
                    # square pass carries Sum(w^2) for [0:sqa]; scratch
                    # lands in the z half (overwritten by ts-z later).
                    # Block 0's ACT share is smaller (its tail chunk goes
                    # to DVE straight from f32, so stats trail the DMA).
                    sqa = (K // 2 + 4096) if b == 0 else SQA
                    hi = min(sqa, base + K // 2) - base
                    if hi > 0:
                        if b == 0:
                            for lo, hi2 in zip(cuts[:-1], cuts[1:]):
                                if lo >= hi:
                                    break
                                nc.scalar.activation(
                                    zh[h][:, lo:min(hi2, hi)],
                                    w32[:, lo:min(hi2, hi)], Act.Square,
                                    accum_out=st[:, 6 + qi:7 + qi])
                                qi += 1
                        else:
                            nc.scalar.activation(
                                zh[h][:, 0:hi], w32[:, 0:hi], Act.Square,
                                accum_out=st[:, 6 + qi:7 + qi])
                            qi += 1
                # remainder of Sum(w^2) on DVE: for block 0 straight from
                # the f32 tail chunk (starts the moment its DMA lands, on
                # an otherwise-idle DVE); for later blocks from w_f16
                if b == 0:
                    nc.vector._custom_dve(sqs,
                                          out=z2[:, 4096:K // 2],
                                          in0=w32[:, 4096:K // 2],
                                          accum_out=st[:, 11:12])
                else:
                    nc.vector._custom_dve(sqs,
                                          out=z2[:, SQA - K // 2:K // 2],
                                          in0=wh[1][:, SQA - K // 2:K // 2],
                                          accum_out=st[:, 11:12])

                # per-channel params: r = 1/step, nb1 = mean*r - 7
                SUQ = minis.tile([P, 2], f32, tag="SUQ")
                nc.vector.tensor_reduce(SUQ[:, 0:1], st[:, 0:si],
                                        mybir.AxisListType.X, Alu.add)
                nc.vector.tensor_reduce(SUQ[:, 1:2], st[:, 6:6 + qi],
                                        mybir.AxisListType.X, Alu.add)
                nc.vector.tensor_tensor(SUQ[:, 1:2], SUQ[:, 1:2],
                                        st[:, 11:12], Alu.add)
                me2 = minis.tile([P, 2], f32, tag="me2")
                nc.vector.tensor_scalar(me2[:], SUQ[:], INV_K, None,
                                        Alu.mult)
                mean = me2[:, 0:1]
                E2 = me2[:, 1:2]
                nvar = minis.tile([P, 1], f32, tag="nvar")
                # nvar = mean*mean - E2  (= -var_biased)
                nc.vector.scalar_tensor_tensor(nvar[:], mean, mean,
                                               E2, Alu.mult, Alu.subtract)
                step = minis.tile([P, 1], f32, tag="step")
                # step = sqrt(K2*var_b) = Sqrt(-K2 * nvar)
                nc.scalar.activation(step[:], nvar[:], Act.Sqrt,
                                     bias=0.0, scale=-K2)
                r = minis.tile([P, 1], f32, tag="r")
                nc.vector.reciprocal(r[:], step[:])
                nb1 = minis.tile([P, 1], f32, tag="nb1")
                # nb1 = mean*r - 7
                nc.vector.scalar_tensor_tensor(nb1[:], mean, r[:],
                                               seven[:], Alu.mult,
                                               Alu.subtract)
                state[b] = (sblk, wh, zh, r, nb1)

            def phase2(b):
                """z pass + fused loss; accum rides the custom op."""
                sblk, wh, zh, r, nb1 = state.pop(b)
                am = minis.tile([P, 2], f32, tag="am")
                for h in range(2):
                    nc.vector.tensor_scalar(zh[h][:], wh[h][:], r[:],
                                            nb1[:], Alu.mult, Alu.subtract)
                    nc.vector._custom_dve(vq, out=zh[h][:], in0=zh[h][:],
                                          in1=wh[h][:],
                                          s0=RND, s1=sblk[:], imm2=RND7,
                                          accum_out=am[:, h:h + 1])
                nc.vector.tensor_reduce(out_sb[:, b:b + 1], am[:],
                                        mybir.AxisListType.X, Alu.add)

            # software pipelining: emit phase1(b+1) before phase2(b) so
            # block b+1's stats minis overlap block b's custom ops on DVE
            phase1(0)
            for b in range(1, NBLK):
                phase1(b)
                phase2(b - 1)
            phase2(NBLK - 1)

            nc.sync.dma_start(out_ext[:], out_sb[:])

    nc.compile()
    return nc


def _get_program():
    global _PROGRAM
    if _PROGRAM is None:
        _PROGRAM = _build()
    return _PROGRAM


def kernel(weight, scale):
    w = np.ascontiguousarray(np.asarray(weight, dtype=np.float32))
    s = np.ascontiguousarray(
        np.asarray(scale, dtype=np.float32)).reshape(CFULL, 1)
    assert w.shape == (CFULL, K), w.shape

    nc = _get_program()
    in_maps = [
        {"w": w[i * CSH:(i + 1) * CSH], "s": s[i * CSH:(i + 1) * CSH]}
        for i in range(NCORES)
    ]
    res = run_bass_kernel_spmd(nc, in_maps, list(range(NCORES)))
    total = 0.0
    for i in range(NCORES):
        total += res.results[i]["out"].astype(np.float64).sum()
    return np.float32(total)
